# revision 30
# baseline (speedup 1.0000x reference)
"""Trainium2 Bass kernel for nn_AttentionBlock (B=8, C=256, T=4096, CQK=32).

Data-parallel over batch: one batch element per NeuronCore (8 cores).
Weights are replicated; each core computes a full attention block for its
batch and the host stacks the per-core outputs.

Per-core algorithm (all matmuls in fp32r = FP22, full PE rate):
  q4/k4 = Wq/Wk projections, replicated 4x across partition quadrants
          (enables 4x row-tiled K=32 QK^T matmuls)
  vT_aug[s, c] = (Wv x)^T with an appended ones column (col 256) so the
          PV matmul produces softmax denominators for free
  scoresT[s, t] = k^T q  (row-tiled, 4 s-chunks -> 4 PSUM banks at once)
  eT = exp(scoresT)      (ACT engine, PSUM -> SBUF, no max subtraction:
                          |scores| <~ 35 so exp stays in fp32 range)
  outT_aug[t, 0:258] = sum_s eT[s,t] * vT_aug[s,:]   (PSUM accumulation)
  out[t, c] = outT[t, c] / outT[t, 256]; transpose to [c, t] on the PE
  final[c, t] = gamma*(out + bv[c]) + x[c, t]
"""

import numpy as np

import concourse.bass as bass
import concourse.bacc as bacc
import concourse.mybir as mybir
import concourse.tile as tile
from concourse.masks import make_identity
from concourse import bass_utils

import os

f32 = mybir.dt.float32
f32r = mybir.dt.float32r
bf16 = mybir.dt.bfloat16
# PV matmul operand dtype: fp32r (FP22) by default; bf16 enables FWL
PV_DT = bf16 if os.environ.get("PV_BF16") else f32r
Exp = mybir.ActivationFunctionType.Exp
Mult = mybir.AluOpType.mult
Add = mybir.AluOpType.add
ts = bass.ts

B, C, T = 8, 256, 4096
CQK = 32
N_CORES = 8
NB = T // 512    # 8 t-blocks of 512
NS = T // 128    # 32 s-chunks of 128
VSTRIDE = 258    # vT chunk stride: 256 v-cols + ones col + pad (fp32r needs even N)


def build_kernel_body(tc: tile.TileContext, out_ap, x_ap, wq_ap, bq_ap, wk_ap,
                      bk_ap, wv_ap, bv_ap, g_ap, repeats: int = 1):
    for _rep in range(repeats):
        _build_once(tc, out_ap, x_ap, wq_ap, bq_ap, wk_ap, bk_ap, wv_ap,
                    bv_ap, g_ap)


def _build_once(tc: tile.TileContext, out_ap, x_ap, wq_ap, bq_ap, wk_ap,
                bk_ap, wv_ap, bv_ap, g_ap):
    nc = tc.nc
    with (
        tc.tile_pool(name="const", bufs=1) as const,
        tc.tile_pool(name="big", bufs=1) as big,
        tc.tile_pool(name="work", bufs=4) as work,
        tc.tile_pool(name="et", bufs=8) as etp,
        tc.tile_pool(name="ps", bufs=1, space="PSUM") as psp,
        tc.tile_pool(name="psa", bufs=4, space="PSUM") as psa,
    ):
        ident = const.tile([128, 128], f32, tag="ident")
        make_identity(nc, ident[:])
        # f32r copy of the identity: fp32r transposes run 1.5 cyc/row vs
        # fp32's 2.0 (memset can't write f32r, so copy from the f32 one)
        ident_r = const.tile([128, 128], f32r, tag="ident_r")
        nc.vector.tensor_copy(ident_r[:], ident[:])
        ones_f32 = const.tile([128, 2], f32, tag="ones")
        nc.vector.memset(ones_f32[:], 1.0)

        # ---- load raw weights/biases ----
        wq_raw = const.tile([CQK, C], f32, tag="wq_raw")
        nc.sync.dma_start(wq_raw[:], wq_ap)
        wk_raw = const.tile([CQK, C], f32, tag="wk_raw")
        nc.sync.dma_start(wk_raw[:], wk_ap)
        wv_raw = []
        for j in range(2):
            t_ = const.tile([128, C], f32, tag=f"wv_raw{j}")
            nc.sync.dma_start(t_[:], wv_ap[ts(j, 128), :])
            wv_raw.append(t_)
        brow = const.tile([1, 2 * CQK + C + 2], f32, tag="brow")
        nc.sync.dma_start(brow[:, 0:CQK], bq_ap.unsqueeze(0))
        nc.sync.dma_start(brow[:, CQK:2 * CQK], bk_ap.unsqueeze(0))
        nc.sync.dma_start(brow[:, 2 * CQK:2 * CQK + C], bv_ap.unsqueeze(0))
        nc.sync.dma_start(brow[:, 2 * CQK + C:2 * CQK + C + 1], g_ap.unsqueeze(0))
        nc.sync.dma_start(brow[:, 2 * CQK + C + 1:2 * CQK + C + 2], g_ap.unsqueeze(0))

        # ---- x (per-core batch): n-outer so projection n=0 starts early ----
        # The DRAM tensor is fp32 (exact residual); the fp32r copy used as
        # matmul input is produced on-device via DVE copies (which round).
        xs = [big.tile([128, T], f32r, tag=f"x{cc}", name=f"x{cc}")
              for cc in range(2)]
        for n in range(NB):
            for cc in range(2):
                xld = work.tile([128, 512], f32, tag="xld")
                nc.sync.dma_start(xld[:], x_ap[ts(cc, 128), ts(n, 512)])
                nc.vector.tensor_copy(xs[cc][:, ts(n, 512)], xld[:])

        # ---- transpose weights; replicate Wq^T/Wk^T 4x along columns ----
        wq4T, wk4T, wvT = [], [], []
        for kc in range(2):
            tq = const.tile([128, 128], f32r, tag=f"wq4T{kc}")
            tk = const.tile([128, 128], f32r, tag=f"wk4T{kc}")
            tv = const.tile([128, C], f32r, tag=f"wvT{kc}")
            for (src, dst) in ((wq_raw, tq), (wk_raw, tk)):
                pt = psa.tile([128, CQK], f32, tag="acc")
                nc.tensor.transpose(pt[:], src[:, ts(kc, 128)], ident[0:CQK, 0:CQK])
                for g in range(4):
                    nc.vector.tensor_copy(dst[:, ts(g, CQK)], pt[:])
            for j in range(2):
                pt = psa.tile([128, 128], f32, tag="acc")
                nc.tensor.transpose(pt[:], wv_raw[j][:, ts(kc, 128)], ident[:])
                nc.vector.tensor_copy(tv[:, ts(j, 128)], pt[:])
            wq4T.append(tq)
            wk4T.append(tk)
            wvT.append(tv)

        # ---- bias columns ----
        # bq4/bk4: [128,1] with bias replicated across the 4 quadrants
        bq4 = const.tile([128, 1], f32, tag="bq4")
        bk4 = const.tile([128, 1], f32, tag="bk4")
        bv2 = []
        pt = psa.tile([CQK, 1], f32, tag="acc")
        nc.tensor.transpose(pt[:], brow[0:1, 0:CQK], ident[0:1, 0:1])
        for g in range(4):
            nc.vector.tensor_copy(bq4[ts(g, CQK), :], pt[:])
        pt = psa.tile([CQK, 1], f32, tag="acc")
        nc.tensor.transpose(pt[:], brow[0:1, CQK:2 * CQK], ident[0:1, 0:1])
        for g in range(4):
            nc.vector.tensor_copy(bk4[ts(g, CQK), :], pt[:])
        for cc in range(2):
            pt = psa.tile([128, 1], f32, tag="acc")
            nc.tensor.transpose(
                pt[:], brow[0:1, 2 * CQK + 128 * cc: 2 * CQK + 128 * (cc + 1)],
                ident[0:1, 0:1])
            bt = const.tile([128, 1], f32, tag=f"bv{cc}")
            nc.vector.tensor_copy(bt[:], pt[:])
            bv2.append(bt)
        # gamma broadcast to [128,1]: outer product ones[1,128] x gamma[1,2]
        grow = const.tile([1, 2], f32r, tag="grow")
        nc.vector.tensor_copy(grow[:], brow[0:1, 2 * CQK + C:2 * CQK + C + 2])
        ones_row_f = const.tile([1, 128], f32, tag="ones_row_f")
        nc.vector.memset(ones_row_f[:], 1.0)
        ones_row_r = const.tile([1, 128], f32r, tag="ones_row_r")
        nc.vector.tensor_copy(ones_row_r[:], ones_row_f[:])
        pt = psa.tile([128, 2], f32, tag="acc")
        nc.tensor.matmul(pt[:], lhsT=ones_row_r[:], rhs=grow[:],
                         start=True, stop=True)
        gam = const.tile([128, 1], f32, tag="gam")
        nc.vector.tensor_copy(gam[:], pt[:, 0:1])
        gbv = []
        for cc in range(2):
            t_ = const.tile([128, 1], f32, tag=f"gbv{cc}")
            nc.vector.tensor_scalar_mul(t_[:], bv2[cc][:], gam[:, 0:1])
            gbv.append(t_)

        # ---- projections: q4/k4 [128, T] (4 copies across quadrants) ----
        q4 = big.tile([128, T], f32r, tag="q4")
        k4 = big.tile([128, T], f32r, tag="k4")
        for n in range(NB):
            qp = psa.tile([128, 512], f32, tag="acc")
            kp = psa.tile([128, 512], f32, tag="acc")
            for kc in range(2):
                nc.tensor.matmul(qp[:], lhsT=wq4T[kc][:], rhs=xs[kc][:, ts(n, 512)],
                                 start=(kc == 0), stop=(kc == 1))
                nc.tensor.matmul(kp[:], lhsT=wk4T[kc][:], rhs=xs[kc][:, ts(n, 512)],
                                 start=(kc == 0), stop=(kc == 1))
            nc.vector.tensor_scalar_add(q4[:, ts(n, 512)], qp[:], bq4[:, 0:1])
            nc.vector.tensor_scalar_add(k4[:, ts(n, 512)], kp[:], bk4[:, 0:1])

        # ---- vT_aug [128, NS*VSTRIDE]: chunks of [s=128, 256 v + ones] ----
        # Emitted inside the first LAG rounds of the main loop (below) so the
        # vT matmuls overlap the first exps instead of serializing before
        # them. All vT PSUM use must finish before the first outT allocation
        # (PV pass 0), hence the PV stagger depth LAG.
        vt = big.tile([128, NS * VSTRIDE], PV_DT, tag="vt")

        def emit_vt_chunk(sb):
            vp = psa.tile([128, C], f32, tag="acc", name=f"vp{sb}")
            for kc in range(2):
                nc.tensor.matmul(vp[:], lhsT=xs[kc][:, ts(sb, 128)], rhs=wvT[kc][:],
                                 start=(kc == 0), stop=(kc == 1))
            nc.vector.tensor_copy(vt[:, sb * VSTRIDE: sb * VSTRIDE + C], vp[:])
            nc.vector.tensor_copy(
                vt[:, sb * VSTRIDE + C: sb * VSTRIDE + C + 2], ones_f32[:, 0:2])

        # ---- main loop: 128 half-rounds (8 t-blocks x 16 halves) ----
        # Each half computes 2 s-chunks of scoresT into a 2-bank staging
        # tile; "stage" has bufs=2 so QKT(h+1) fills one tile while ACT
        # exps the other — breaking the exp->QKT serialization a single
        # 4-bank stage would force. PV lags by LAG halves; the first LAG
        # halves are free of outT PSUM pressure so the vT build can run.
        LAG = 6
        vt_per_half = -(-NS // LAG)  # ceil
        et_tiles = {}
        outT = None
        NH = NB * 16
        for r in range(NH + LAG):
            if r < NH:
                j, i2 = divmod(r, 16)
                st = psp.tile([128, 1024], f32, tag="stage", bufs=2)
                for l in range(2):
                    sc = 2 * i2 + l
                    g = sc % 4
                    nc.tensor.matmul(
                        st[:, ts(l, 512)],
                        lhsT=k4[ts(g, CQK), ts(sc, 128)],
                        rhs=q4[ts(g, CQK), ts(j, 512)],
                        start=True, stop=True,
                        tile_position=(32 * g, 0),
                    )
                et = etp.tile([128, 1024], PV_DT, tag="eT")
                nc.scalar.activation(et[:], st[:], Exp)
                et_tiles[r] = et
            if r < LAG:
                for sb in range(r * vt_per_half,
                                min((r + 1) * vt_per_half, NS)):
                    emit_vt_chunk(sb)
            if r >= LAG:
                jp, ip = divmod(r - LAG, 16)
                et = et_tiles.pop(r - LAG)
                if ip == 0:
                    outT = [psa.tile([128, VSTRIDE], f32, tag="acc",
                                     name=f"outT_{jp}_{_jj}")
                            for _jj in range(4)]
                for l in range(2):
                    sc = 2 * ip + l
                    for jj in range(4):
                        nc.tensor.matmul(
                            outT[jj][:],
                            lhsT=et[:, l * 512 + jj * 128: l * 512 + (jj + 1) * 128],
                            rhs=vt[:, sc * VSTRIDE: (sc + 1) * VSTRIDE],
                            start=(sc == 0), stop=(sc == NS - 1),
                        )
                if ip == 15:
                    # epilogue for block jp
                    fin = [work.tile([128, 512], f32, tag=f"fin{cc}",
                                     name=f"fin_{jp}_{cc}")
                           for cc in range(2)]
                    for jj in range(4):
                        rec = work.tile([128, 1], f32, tag="rec")
                        nc.vector.reciprocal(rec[:], outT[jj][:, C:C + 1])
                        osb = work.tile([128, C], f32r, tag="osb")
                        nc.vector.tensor_scalar_mul(
                            osb[:], outT[jj][:, 0:C], rec[:, 0:1])
                        for cc in range(2):
                            tp = psa.tile([128, 128], f32r, tag="acc")
                            nc.tensor.transpose(tp[:], osb[:, ts(cc, 128)],
                                                ident_r[:])
                            nc.vector.tensor_scalar(
                                fin[cc][:, ts(jj, 128)], tp[:].bitcast(f32),
                                gam[:, 0:1], gbv[cc][:, 0:1], Mult, Add)
                    for cc in range(2):
                        xres = work.tile([128, 512], f32, tag="xres")
                        nc.sync.dma_start(xres[:], x_ap[ts(cc, 128), ts(jp, 512)])
                        dma_t = work.tile([128, 512], f32, tag=f"dma{cc}")
                        nc.vector.tensor_add(dma_t[:], fin[cc][:], xres[:])
                        nc.sync.dma_start(
                            out_ap[ts(cc, 128), ts(jp, 512)], dma_t[:])


_STATE = None


def _build_nc(repeats: int = 1):
    nc = bacc.Bacc("TRN2", debug=False, num_devices=N_CORES)
    x_t = nc.dram_tensor("x", [C, T], f32, kind="ExternalInput")
    wq_t = nc.dram_tensor("wq", [CQK, C], f32, kind="ExternalInput")
    bq_t = nc.dram_tensor("bq", [CQK], f32, kind="ExternalInput")
    wk_t = nc.dram_tensor("wk", [CQK, C], f32, kind="ExternalInput")
    bk_t = nc.dram_tensor("bk", [CQK], f32, kind="ExternalInput")
    wv_t = nc.dram_tensor("wv", [C, C], f32, kind="ExternalInput")
    bv_t = nc.dram_tensor("bv", [C], f32, kind="ExternalInput")
    g_t = nc.dram_tensor("gamma", [1], f32, kind="ExternalInput")
    out_t = nc.dram_tensor("out", [C, T], f32, kind="ExternalOutput")
    with tile.TileContext(nc) as tc:
        build_kernel_body(tc, out_t.ap(), x_t.ap(), wq_t.ap(), bq_t.ap(),
                          wk_t.ap(), bk_t.ap(), wv_t.ap(), bv_t.ap(), g_t.ap(),
                          repeats=repeats)
    nc.compile()
    return nc


class _Executor:
    """Persistent jitted shard_map executor (modeled on run_bass_via_pjrt)."""

    def __init__(self, nc, donate=True):
        import jax
        from jax.experimental.shard_map import shard_map
        from jax.sharding import Mesh, PartitionSpec
        from concourse import bass2jax

        bass2jax.install_neuronx_cc_hook()
        self.nc = nc
        in_names, out_names, out_avals, zero_outs = [], [], [], []
        pname = nc.partition_id_tensor.name if nc.partition_id_tensor else None
        for alloc in nc.m.functions[0].allocations:
            if not isinstance(alloc, mybir.MemoryLocationSet):
                continue
            name = alloc.memorylocations[0].name
            if alloc.kind == "ExternalInput":
                if name != pname:
                    in_names.append(name)
            elif alloc.kind == "ExternalOutput":
                shape = tuple(alloc.tensor_shape)
                dtype = mybir.dt.np(alloc.dtype)
                out_names.append(name)
                out_avals.append(jax.core.ShapedArray(shape, dtype))
                zero_outs.append(np.zeros(shape, dtype))
        self.in_names = list(in_names)
        self.out_names = out_names
        self.out_avals = out_avals
        self.zero_outs = zero_outs
        n_params = len(in_names)
        n_outs = len(out_names)
        all_in = in_names + out_names
        if pname is not None:
            all_in.append(pname)

        def _body(*args):
            operands = list(args)
            if pname is not None:
                operands.append(bass2jax.partition_id_tensor())
            outs = bass2jax._bass_exec_p.bind(
                *operands,
                out_avals=tuple(out_avals),
                in_names=tuple(all_in),
                out_names=tuple(out_names),
                lowering_input_output_aliases=(),
                sim_require_finite=True,
                sim_require_nnan=True,
                nc=nc,
            )
            return tuple(outs)

        devices = jax.devices()[:N_CORES]
        mesh = Mesh(np.asarray(devices), ("core",))
        in_specs = (PartitionSpec("core"),) * (n_params + n_outs)
        out_specs = (PartitionSpec("core"),) * n_outs
        self.fn = jax.jit(
            shard_map(_body, mesh=mesh, in_specs=in_specs, out_specs=out_specs,
                      check_rep=False),
            donate_argnums=(tuple(range(n_params, n_params + n_outs))
                            if donate else ()),
            keep_unused=True,
        )

    def concat_inputs(self, in_maps):
        return [np.concatenate([np.asarray(m[name]) for m in in_maps], axis=0)
                for name in self.in_names]

    def zeros(self):
        return [np.zeros((N_CORES * z.shape[0], *z.shape[1:]), z.dtype)
                for z in self.zero_outs]

    def run(self, concat_in):
        return self.fn(*concat_in, *self.zeros())

    def make_chain_fn(self, k_iters):
        """Jitted function that runs the NEFF k_iters times back-to-back on
        device, feeding each iteration's output back in as x (with gamma=0
        the output equals x, so every iteration does identical work). Used
        to measure per-execution device time without host/transfer overhead.
        """
        import jax
        import jax.numpy as jnp
        from jax.experimental.shard_map import shard_map
        from jax.sharding import Mesh, PartitionSpec
        from concourse import bass2jax

        nc = self.nc
        pname = nc.partition_id_tensor.name if nc.partition_id_tensor else None
        all_in = self.in_names + self.out_names
        if pname is not None:
            all_in.append(pname)
        out_avals = self.out_avals
        n_params = len(self.in_names)

        def chain_body(*args):
            x_loc = args[0]
            rest = args[1:n_params]
            zeros = args[n_params:]
            for _ in range(k_iters):
                operands = [x_loc, *rest, *zeros]
                if pname is not None:
                    operands.append(bass2jax.partition_id_tensor())
                outs = bass2jax._bass_exec_p.bind(
                    *operands,
                    out_avals=tuple(out_avals),
                    in_names=tuple(all_in),
                    out_names=tuple(self.out_names),
                    lowering_input_output_aliases=(),
                    sim_require_finite=True,
                    sim_require_nnan=True,
                    nc=nc,
                )
                x_loc = outs[0]
            return (x_loc,)

        devices = jax.devices()[:N_CORES]
        mesh = Mesh(np.asarray(devices), ("core",))
        n_outs = len(self.out_names)
        return jax.jit(
            shard_map(chain_body, mesh=mesh,
                      in_specs=(PartitionSpec("core"),) * (n_params + n_outs),
                      out_specs=(PartitionSpec("core"),),
                      check_rep=False),
            keep_unused=True,
        )


def _get_executor():
    global _STATE
    if _STATE is None:
        _STATE = _Executor(_build_nc())
    return _STATE


def _in_maps(x, Wq, bq, Wk, bk, Wv, bv, gamma):
    f = lambda a: np.ascontiguousarray(np.asarray(a, dtype=np.float32))
    shared = {"wq": f(Wq), "bq": f(bq), "wk": f(Wk), "bk": f(bk),
              "wv": f(Wv), "bv": f(bv), "gamma": f(gamma)}
    return [{"x": f(x[b]), **shared} for b in range(B)]


def kernel(x, Wq, bq, Wk, bk, Wv, bv, gamma):
    global _STATE
    in_maps = _in_maps(x, Wq, bq, Wk, bk, Wv, bv, gamma)
    for attempt in range(2):
        try:
            ex = _get_executor()
            outs = ex.run(ex.concat_inputs(in_maps))
            return np.asarray(outs[0]).reshape(B, C, T).astype(np.float32)
        except Exception:
            if attempt == 1:
                raise
            _STATE = None  # transient device error: rebuild and retry once


if __name__ == "__main__":
    rng = np.random.default_rng(0)
    x = rng.standard_normal((B, C, T), dtype=np.float32)
    Wq = (rng.standard_normal((CQK, C), dtype=np.float32) / 16)
    bq = rng.standard_normal(CQK).astype(np.float32) * 0.02
    Wk = (rng.standard_normal((CQK, C), dtype=np.float32) / 16)
    bk = rng.standard_normal(CQK).astype(np.float32) * 0.02
    Wv = (rng.standard_normal((C, C), dtype=np.float32) / 16)
    bv = rng.standard_normal(C).astype(np.float32) * 0.02
    gamma = np.ones(1, dtype=np.float32)
    out = kernel(x, Wq, bq, Wk, bk, Wv, bv, gamma)
    # numpy reference
    q = np.einsum("oc,bct->bot", Wq, x) + bq[None, :, None]
    k = np.einsum("oc,bct->bot", Wk, x) + bk[None, :, None]
    v = np.einsum("oc,bct->bot", Wv, x) + bv[None, :, None]
    s = np.einsum("bot,bos->bts", q, k)
    s = s - s.max(-1, keepdims=True)
    e = np.exp(s)
    a = e / e.sum(-1, keepdims=True)
    ref = gamma[0] * np.einsum("bcs,bts->bct", v, a) + x
    err = np.abs(out - ref).max() / np.abs(ref).max()
    print("max rel err vs numpy (gamma=1):", err)


# revision 33
# speedup vs baseline: 1.0228x; 1.0228x over previous
"""Trainium2 Bass kernel for nn_AttentionBlock (B=8, C=256, T=4096, CQK=32).

Data-parallel over batch: one batch element per NeuronCore (8 cores).
Weights are replicated; each core computes a full attention block for its
batch and the host stacks the per-core outputs.

Per-core algorithm (all matmuls in fp32r = FP22, full PE rate):
  q4/k4 = Wq/Wk projections, replicated 4x across partition quadrants
          (enables 4x row-tiled K=32 QK^T matmuls)
  vT_aug[s, c] = (Wv x)^T with an appended ones column (col 256) so the
          PV matmul produces softmax denominators for free
  scoresT[s, t] = k^T q  (row-tiled, 4 s-chunks -> 4 PSUM banks at once)
  eT = exp(scoresT)      (ACT engine, PSUM -> SBUF, no max subtraction:
                          |scores| <~ 35 so exp stays in fp32 range)
  outT_aug[t, 0:258] = sum_s eT[s,t] * vT_aug[s,:]   (PSUM accumulation)
  out[t, c] = outT[t, c] / outT[t, 256]; transpose to [c, t] on the PE
  final[c, t] = gamma*(out + bv[c]) + x[c, t]
"""

import numpy as np

import concourse.bass as bass
import concourse.bacc as bacc
import concourse.mybir as mybir
import concourse.tile as tile
from concourse.masks import make_identity
from concourse import bass_utils

import os

f32 = mybir.dt.float32
f32r = mybir.dt.float32r
bf16 = mybir.dt.bfloat16
# PV matmul operand dtype: fp32r (FP22) by default; bf16 enables FWL
PV_DT = bf16 if os.environ.get("PV_BF16") else f32r
Exp = mybir.ActivationFunctionType.Exp
Mult = mybir.AluOpType.mult
Add = mybir.AluOpType.add
ts = bass.ts

B, C, T = 8, 256, 4096
CQK = 32
N_CORES = 8
NB = T // 512    # 8 t-blocks of 512
NS = T // 128    # 32 s-chunks of 128
VSTRIDE = 258    # vT chunk stride: 256 v-cols + ones col + pad (fp32r needs even N)


def build_kernel_body(tc: tile.TileContext, out_ap, x_ap, wq_ap, bq_ap, wk_ap,
                      bk_ap, wv_ap, bv_ap, g_ap, repeats: int = 1):
    for _rep in range(repeats):
        _build_once(tc, out_ap, x_ap, wq_ap, bq_ap, wk_ap, bk_ap, wv_ap,
                    bv_ap, g_ap)


def _build_once(tc: tile.TileContext, out_ap, x_ap, wq_ap, bq_ap, wk_ap,
                bk_ap, wv_ap, bv_ap, g_ap):
    nc = tc.nc
    with (
        tc.tile_pool(name="const", bufs=1) as const,
        tc.tile_pool(name="big", bufs=1) as big,
        tc.tile_pool(name="work", bufs=4) as work,
        tc.tile_pool(name="et", bufs=5) as etp,
        tc.tile_pool(name="ps", bufs=1, space="PSUM") as psp,
        tc.tile_pool(name="psa", bufs=4, space="PSUM") as psa,
    ):
        ident = const.tile([128, 128], f32, tag="ident")
        make_identity(nc, ident[:])
        # f32r copy of the identity: fp32r transposes run 1.5 cyc/row vs
        # fp32's 2.0 (memset can't write f32r, so copy from the f32 one)
        ident_r = const.tile([128, 128], f32r, tag="ident_r")
        nc.vector.tensor_copy(ident_r[:], ident[:])
        ones_f32 = const.tile([128, 2], f32, tag="ones")
        nc.vector.memset(ones_f32[:], 1.0)

        # ---- load raw weights/biases ----
        wq_raw = const.tile([CQK, C], f32, tag="wq_raw")
        nc.sync.dma_start(wq_raw[:], wq_ap)
        wk_raw = const.tile([CQK, C], f32, tag="wk_raw")
        nc.sync.dma_start(wk_raw[:], wk_ap)
        wv_raw = []
        for j in range(2):
            t_ = const.tile([128, C], f32, tag=f"wv_raw{j}")
            nc.sync.dma_start(t_[:], wv_ap[ts(j, 128), :])
            wv_raw.append(t_)
        brow = const.tile([1, 2 * CQK + C + 2], f32, tag="brow")
        nc.sync.dma_start(brow[:, 0:CQK], bq_ap.unsqueeze(0))
        nc.sync.dma_start(brow[:, CQK:2 * CQK], bk_ap.unsqueeze(0))
        nc.sync.dma_start(brow[:, 2 * CQK:2 * CQK + C], bv_ap.unsqueeze(0))
        nc.sync.dma_start(brow[:, 2 * CQK + C:2 * CQK + C + 1], g_ap.unsqueeze(0))
        nc.sync.dma_start(brow[:, 2 * CQK + C + 1:2 * CQK + C + 2], g_ap.unsqueeze(0))

        # ---- x (per-core batch): n-outer so projection n=0 starts early ----
        # The DRAM tensor is fp32 (exact residual); the fp32r copy used as
        # matmul input is produced on-device via DVE copies (which round).
        xs = [big.tile([128, T], f32r, tag=f"x{cc}", name=f"x{cc}")
              for cc in range(2)]
        for n in range(NB):
            for cc in range(2):
                xld = work.tile([128, 512], f32, tag="xld")
                nc.sync.dma_start(xld[:], x_ap[ts(cc, 128), ts(n, 512)])
                nc.vector.tensor_copy(xs[cc][:, ts(n, 512)], xld[:])

        # ---- transpose weights; replicate Wq^T/Wk^T 4x along columns ----
        wq4T, wk4T, wvT = [], [], []
        for kc in range(2):
            tq = const.tile([128, 128], f32r, tag=f"wq4T{kc}")
            tk = const.tile([128, 128], f32r, tag=f"wk4T{kc}")
            tv = const.tile([128, C], f32r, tag=f"wvT{kc}")
            for (src, dst) in ((wq_raw, tq), (wk_raw, tk)):
                pt = psa.tile([128, CQK], f32, tag="acc")
                nc.tensor.transpose(pt[:], src[:, ts(kc, 128)], ident[0:CQK, 0:CQK])
                for g in range(4):
                    nc.vector.tensor_copy(dst[:, ts(g, CQK)], pt[:])
            for j in range(2):
                pt = psa.tile([128, 128], f32, tag="acc")
                nc.tensor.transpose(pt[:], wv_raw[j][:, ts(kc, 128)], ident[:])
                nc.vector.tensor_copy(tv[:, ts(j, 128)], pt[:])
            wq4T.append(tq)
            wk4T.append(tk)
            wvT.append(tv)

        # ---- bias columns ----
        # bq4/bk4: [128,1] with bias replicated across the 4 quadrants
        bq4 = const.tile([128, 1], f32, tag="bq4")
        bk4 = const.tile([128, 1], f32, tag="bk4")
        bv2 = []
        pt = psa.tile([CQK, 1], f32, tag="acc")
        nc.tensor.transpose(pt[:], brow[0:1, 0:CQK], ident[0:1, 0:1])
        for g in range(4):
            nc.vector.tensor_copy(bq4[ts(g, CQK), :], pt[:])
        pt = psa.tile([CQK, 1], f32, tag="acc")
        nc.tensor.transpose(pt[:], brow[0:1, CQK:2 * CQK], ident[0:1, 0:1])
        for g in range(4):
            nc.vector.tensor_copy(bk4[ts(g, CQK), :], pt[:])
        for cc in range(2):
            pt = psa.tile([128, 1], f32, tag="acc")
            nc.tensor.transpose(
                pt[:], brow[0:1, 2 * CQK + 128 * cc: 2 * CQK + 128 * (cc + 1)],
                ident[0:1, 0:1])
            bt = const.tile([128, 1], f32, tag=f"bv{cc}")
            nc.vector.tensor_copy(bt[:], pt[:])
            bv2.append(bt)
        # gamma broadcast to [128,1]: outer product ones[1,128] x gamma[1,2]
        grow = const.tile([1, 2], f32r, tag="grow")
        nc.vector.tensor_copy(grow[:], brow[0:1, 2 * CQK + C:2 * CQK + C + 2])
        ones_row_f = const.tile([1, 128], f32, tag="ones_row_f")
        nc.vector.memset(ones_row_f[:], 1.0)
        ones_row_r = const.tile([1, 128], f32r, tag="ones_row_r")
        nc.vector.tensor_copy(ones_row_r[:], ones_row_f[:])
        pt = psa.tile([128, 2], f32, tag="acc")
        nc.tensor.matmul(pt[:], lhsT=ones_row_r[:], rhs=grow[:],
                         start=True, stop=True)
        gam = const.tile([128, 1], f32, tag="gam")
        nc.vector.tensor_copy(gam[:], pt[:, 0:1])
        gbv = []
        for cc in range(2):
            t_ = const.tile([128, 1], f32, tag=f"gbv{cc}")
            nc.vector.tensor_scalar_mul(t_[:], bv2[cc][:], gam[:, 0:1])
            gbv.append(t_)

        # ---- projections: q4/k4 [128, T] (4 copies across quadrants) ----
        q4 = big.tile([128, T], f32r, tag="q4")
        k4 = big.tile([128, T], f32r, tag="k4")
        for n in range(NB):
            qp = psa.tile([128, 512], f32, tag="acc")
            kp = psa.tile([128, 512], f32, tag="acc")
            for kc in range(2):
                nc.tensor.matmul(qp[:], lhsT=wq4T[kc][:], rhs=xs[kc][:, ts(n, 512)],
                                 start=(kc == 0), stop=(kc == 1))
                nc.tensor.matmul(kp[:], lhsT=wk4T[kc][:], rhs=xs[kc][:, ts(n, 512)],
                                 start=(kc == 0), stop=(kc == 1))
            nc.vector.tensor_scalar_add(q4[:, ts(n, 512)], qp[:], bq4[:, 0:1])
            nc.vector.tensor_scalar_add(k4[:, ts(n, 512)], kp[:], bk4[:, 0:1])

        # ---- vT_aug [128, NS*VSTRIDE]: chunks of [s=128, 256 v + ones] ----
        # Emitted inside the first LAG rounds of the main loop (below) so the
        # vT matmuls overlap the first exps instead of serializing before
        # them. All vT PSUM use must finish before the first outT allocation
        # (PV pass 0), hence the PV stagger depth LAG.
        vt = big.tile([128, NS * VSTRIDE], PV_DT, tag="vt")

        def emit_vt_chunk(sb):
            vp = psa.tile([128, C], f32, tag="acc", name=f"vp{sb}")
            for kc in range(2):
                nc.tensor.matmul(vp[:], lhsT=xs[kc][:, ts(sb, 128)], rhs=wvT[kc][:],
                                 start=(kc == 0), stop=(kc == 1))
            nc.vector.tensor_copy(vt[:, sb * VSTRIDE: sb * VSTRIDE + C], vp[:])
            nc.vector.tensor_copy(
                vt[:, sb * VSTRIDE + C: sb * VSTRIDE + C + 2], ones_f32[:, 0:2])

        # ---- main loop: 64 rounds (8 t-blocks x 8 QKT/exp rounds) ----
        # Round r: QKT+exp for round r; PV for round r-LAG; epilogue when a
        # block's last PV pass is emitted. This staggering keeps the PE busy
        # with PV matmuls while ACT runs exp for later rounds, and leaves
        # the first LAG rounds free of outT PSUM pressure for the vT build.
        LAG = 3
        vt_per_round = -(-NS // LAG)  # ceil
        et_tiles = {}
        outT = None
        for r in range(NB * 8 + LAG):
            if r < NB * 8:
                j, i = divmod(r, 8)
                st = psp.tile([128, 2048], f32, tag="stage")
                for g in range(4):
                    sc = 4 * i + g
                    nc.tensor.matmul(
                        st[:, ts(g, 512)],
                        lhsT=k4[ts(g, CQK), ts(sc, 128)],
                        rhs=q4[ts(g, CQK), ts(j, 512)],
                        start=True, stop=True,
                        tile_position=(32 * g, 0),
                    )
                et = etp.tile([128, 2048], PV_DT, tag="eT")
                nc.scalar.activation(et[:], st[:], Exp)
                et_tiles[r] = et
            if r < LAG:
                for sb in range(r * vt_per_round,
                                min((r + 1) * vt_per_round, NS)):
                    emit_vt_chunk(sb)
            if r >= LAG:
                jp, ip = divmod(r - LAG, 8)
                et = et_tiles.pop(r - LAG)
                if ip == 0:
                    outT = [psa.tile([128, VSTRIDE], f32, tag="acc",
                                     name=f"outT_{jp}_{_jj}")
                            for _jj in range(4)]
                for g in range(4):
                    sc = 4 * ip + g
                    for jj in range(4):
                        nc.tensor.matmul(
                            outT[jj][:],
                            lhsT=et[:, g * 512 + jj * 128: g * 512 + (jj + 1) * 128],
                            rhs=vt[:, sc * VSTRIDE: (sc + 1) * VSTRIDE],
                            start=(sc == 0), stop=(sc == NS - 1),
                        )
                if ip == 7:
                    # epilogue for block jp
                    fin = [work.tile([128, 512], f32, tag=f"fin{cc}",
                                     name=f"fin_{jp}_{cc}")
                           for cc in range(2)]
                    for jj in range(4):
                        rec = work.tile([128, 1], f32, tag="rec")
                        nc.vector.reciprocal(rec[:], outT[jj][:, C:C + 1])
                        osb = work.tile([128, C], f32r, tag="osb")
                        nc.vector.tensor_scalar_mul(
                            osb[:], outT[jj][:, 0:C], rec[:, 0:1])
                        for cc in range(2):
                            tp = psa.tile([128, 128], f32r, tag="acc")
                            nc.tensor.transpose(tp[:], osb[:, ts(cc, 128)],
                                                ident_r[:])
                            nc.vector.tensor_scalar(
                                fin[cc][:, ts(jj, 128)], tp[:].bitcast(f32),
                                gam[:, 0:1], gbv[cc][:, 0:1], Mult, Add)
                    for cc in range(2):
                        xres = work.tile([128, 512], f32, tag="xres")
                        nc.sync.dma_start(xres[:], x_ap[ts(cc, 128), ts(jp, 512)])
                        dma_t = work.tile([128, 512], f32, tag=f"dma{cc}")
                        nc.vector.tensor_add(dma_t[:], fin[cc][:], xres[:])
                        nc.sync.dma_start(
                            out_ap[ts(cc, 128), ts(jp, 512)], dma_t[:])


_STATE = None


def _build_nc(repeats: int = 1):
    nc = bacc.Bacc("TRN2", debug=False, num_devices=N_CORES)
    x_t = nc.dram_tensor("x", [C, T], f32, kind="ExternalInput")
    wq_t = nc.dram_tensor("wq", [CQK, C], f32, kind="ExternalInput")
    bq_t = nc.dram_tensor("bq", [CQK], f32, kind="ExternalInput")
    wk_t = nc.dram_tensor("wk", [CQK, C], f32, kind="ExternalInput")
    bk_t = nc.dram_tensor("bk", [CQK], f32, kind="ExternalInput")
    wv_t = nc.dram_tensor("wv", [C, C], f32, kind="ExternalInput")
    bv_t = nc.dram_tensor("bv", [C], f32, kind="ExternalInput")
    g_t = nc.dram_tensor("gamma", [1], f32, kind="ExternalInput")
    out_t = nc.dram_tensor("out", [C, T], f32, kind="ExternalOutput")
    with tile.TileContext(nc) as tc:
        build_kernel_body(tc, out_t.ap(), x_t.ap(), wq_t.ap(), bq_t.ap(),
                          wk_t.ap(), bk_t.ap(), wv_t.ap(), bv_t.ap(), g_t.ap(),
                          repeats=repeats)
    nc.compile()
    return nc


class _Executor:
    """Persistent jitted shard_map executor (modeled on run_bass_via_pjrt)."""

    def __init__(self, nc, donate=True):
        import jax
        from jax.experimental.shard_map import shard_map
        from jax.sharding import Mesh, PartitionSpec
        from concourse import bass2jax

        bass2jax.install_neuronx_cc_hook()
        self.nc = nc
        in_names, out_names, out_avals, zero_outs = [], [], [], []
        pname = nc.partition_id_tensor.name if nc.partition_id_tensor else None
        for alloc in nc.m.functions[0].allocations:
            if not isinstance(alloc, mybir.MemoryLocationSet):
                continue
            name = alloc.memorylocations[0].name
            if alloc.kind == "ExternalInput":
                if name != pname:
                    in_names.append(name)
            elif alloc.kind == "ExternalOutput":
                shape = tuple(alloc.tensor_shape)
                dtype = mybir.dt.np(alloc.dtype)
                out_names.append(name)
                out_avals.append(jax.core.ShapedArray(shape, dtype))
                zero_outs.append(np.zeros(shape, dtype))
        self.in_names = list(in_names)
        self.out_names = out_names
        self.out_avals = out_avals
        self.zero_outs = zero_outs
        n_params = len(in_names)
        n_outs = len(out_names)
        all_in = in_names + out_names
        if pname is not None:
            all_in.append(pname)

        def _body(*args):
            operands = list(args)
            if pname is not None:
                operands.append(bass2jax.partition_id_tensor())
            outs = bass2jax._bass_exec_p.bind(
                *operands,
                out_avals=tuple(out_avals),
                in_names=tuple(all_in),
                out_names=tuple(out_names),
                lowering_input_output_aliases=(),
                sim_require_finite=True,
                sim_require_nnan=True,
                nc=nc,
            )
            return tuple(outs)

        devices = jax.devices()[:N_CORES]
        mesh = Mesh(np.asarray(devices), ("core",))
        in_specs = (PartitionSpec("core"),) * (n_params + n_outs)
        out_specs = (PartitionSpec("core"),) * n_outs
        self.fn = jax.jit(
            shard_map(_body, mesh=mesh, in_specs=in_specs, out_specs=out_specs,
                      check_rep=False),
            donate_argnums=(tuple(range(n_params, n_params + n_outs))
                            if donate else ()),
            keep_unused=True,
        )

    def concat_inputs(self, in_maps):
        return [np.concatenate([np.asarray(m[name]) for m in in_maps], axis=0)
                for name in self.in_names]

    def zeros(self):
        return [np.zeros((N_CORES * z.shape[0], *z.shape[1:]), z.dtype)
                for z in self.zero_outs]

    def run(self, concat_in):
        return self.fn(*concat_in, *self.zeros())

    def make_chain_fn(self, k_iters):
        """Jitted function that runs the NEFF k_iters times back-to-back on
        device, feeding each iteration's output back in as x (with gamma=0
        the output equals x, so every iteration does identical work). Used
        to measure per-execution device time without host/transfer overhead.
        """
        import jax
        import jax.numpy as jnp
        from jax.experimental.shard_map import shard_map
        from jax.sharding import Mesh, PartitionSpec
        from concourse import bass2jax

        nc = self.nc
        pname = nc.partition_id_tensor.name if nc.partition_id_tensor else None
        all_in = self.in_names + self.out_names
        if pname is not None:
            all_in.append(pname)
        out_avals = self.out_avals
        n_params = len(self.in_names)

        def chain_body(*args):
            x_loc = args[0]
            rest = args[1:n_params]
            zeros = args[n_params:]
            for _ in range(k_iters):
                operands = [x_loc, *rest, *zeros]
                if pname is not None:
                    operands.append(bass2jax.partition_id_tensor())
                outs = bass2jax._bass_exec_p.bind(
                    *operands,
                    out_avals=tuple(out_avals),
                    in_names=tuple(all_in),
                    out_names=tuple(self.out_names),
                    lowering_input_output_aliases=(),
                    sim_require_finite=True,
                    sim_require_nnan=True,
                    nc=nc,
                )
                x_loc = outs[0]
            return (x_loc,)

        devices = jax.devices()[:N_CORES]
        mesh = Mesh(np.asarray(devices), ("core",))
        n_outs = len(self.out_names)
        return jax.jit(
            shard_map(chain_body, mesh=mesh,
                      in_specs=(PartitionSpec("core"),) * (n_params + n_outs),
                      out_specs=(PartitionSpec("core"),),
                      check_rep=False),
            keep_unused=True,
        )


def _get_executor():
    global _STATE
    if _STATE is None:
        _STATE = _Executor(_build_nc())
    return _STATE


def _in_maps(x, Wq, bq, Wk, bk, Wv, bv, gamma):
    f = lambda a: np.ascontiguousarray(np.asarray(a, dtype=np.float32))
    shared = {"wq": f(Wq), "bq": f(bq), "wk": f(Wk), "bk": f(bk),
              "wv": f(Wv), "bv": f(bv), "gamma": f(gamma)}
    return [{"x": f(x[b]), **shared} for b in range(B)]


def kernel(x, Wq, bq, Wk, bk, Wv, bv, gamma):
    global _STATE
    in_maps = _in_maps(x, Wq, bq, Wk, bk, Wv, bv, gamma)
    for attempt in range(2):
        try:
            ex = _get_executor()
            outs = ex.run(ex.concat_inputs(in_maps))
            return np.asarray(outs[0]).reshape(B, C, T).astype(np.float32)
        except Exception:
            if attempt == 1:
                raise
            _STATE = None  # transient device error: rebuild and retry once


if __name__ == "__main__":
    rng = np.random.default_rng(0)
    x = rng.standard_normal((B, C, T), dtype=np.float32)
    Wq = (rng.standard_normal((CQK, C), dtype=np.float32) / 16)
    bq = rng.standard_normal(CQK).astype(np.float32) * 0.02
    Wk = (rng.standard_normal((CQK, C), dtype=np.float32) / 16)
    bk = rng.standard_normal(CQK).astype(np.float32) * 0.02
    Wv = (rng.standard_normal((C, C), dtype=np.float32) / 16)
    bv = rng.standard_normal(C).astype(np.float32) * 0.02
    gamma = np.ones(1, dtype=np.float32)
    out = kernel(x, Wq, bq, Wk, bk, Wv, bv, gamma)
    # numpy reference
    q = np.einsum("oc,bct->bot", Wq, x) + bq[None, :, None]
    k = np.einsum("oc,bct->bot", Wk, x) + bk[None, :, None]
    v = np.einsum("oc,bct->bot", Wv, x) + bv[None, :, None]
    s = np.einsum("bot,bos->bts", q, k)
    s = s - s.max(-1, keepdims=True)
    e = np.exp(s)
    a = e / e.sum(-1, keepdims=True)
    ref = gamma[0] * np.einsum("bcs,bts->bct", v, a) + x
    err = np.abs(out - ref).max() / np.abs(ref).max()
    print("max rel err vs numpy (gamma=1):", err)
